# revision 14
# baseline (speedup 1.0000x reference)
"""MultiHeadAttention (B=16, L=1024, D=512, H=8) Trainium2 Bass kernel.

Sharding: data-parallel over batch. Core c computes batches [2c, 2c+1]
fully (projections, attention, fc, layernorm) -- no collectives.

Per-batch dataflow on one core:
  - q/k/v [1024,512] f32 DMA'd in natural layout, PE-transposed to
    qT/kT/vT [512(d) part-chunks, 1024(t)].
  - Projections (float32r matmuls): qhT/khT [512(dh) chunks, 1024] and
    vh [1024(t) chunks, 512(dv)].
  - mask u8 -> maskbias bf16 (-8e9 * mask), kept in [qt,kt] layout and
    PE-transposed to [kt,qt].
  - Per head:
      scores psum[qt,kt] = qhT^T @ khT  (K=64), + maskbias via
        identity-matmul accumulate into PSUM,
      E = exp(0.125*S) on ScalarE with accum_out giving row-sums
        (softmax denominators for free), normalize by reciprocal,
        DMA straight out as `attn`.
      Transposed scores psum[kt,qt] (operands swapped) + maskbiasT,
      ET = exp(0.125*ST); oT[dv,qt] += vh^T-slice @ ET accumulated over
        kt chunks. This avoids transposing the 16.8M-element attn matrix.
  - fc: out[t,dm] = oT_all^T @ Wfc accumulated over dv chunks, + residual
    q, then layernorm (bn_stats/bn_aggr, sqrt+reciprocal). gamma/beta are
    compile-time ones/zeros in this problem => identity, skipped.

All matmuls use float32r (TF32-class single-pass mode, 1 cyc/row for
N>=256 vs 4 for fp32). Data stays f32 in SBUF; APs are bitcast at the
matmul call sites.
"""

import os
import sys

import numpy as np

sys.path.insert(0, "/opt/trn_rl_repo")

from contextlib import ExitStack

import concourse.bass as bass
import concourse.mybir as mybir
import concourse.tile as tile
from concourse import bacc
from concourse.bass_utils import run_bass_kernel_spmd
from concourse.masks import make_identity

P = 128
B, L, D = 16, 1024, 512
H, DH = 8, 64
NCORES = 8
BPC = B // NCORES  # batches per core
TCH = L // P       # 8 token chunks
DCH = D // P       # 4 d_model chunks
NKT = L // P       # 8 kt chunks
HF = 512           # matmul moving half (psum bank)
EPS = 1e-6
SCALE = 1.0 / np.sqrt(DH)  # 0.125
MASKVAL = -8.0e9           # pre-scaled: exp(SCALE*(s + MASKVAL*m)) == 0 when m=1

F32 = mybir.dt.float32
F32R = mybir.dt.float32r
BF16 = mybir.dt.bfloat16
U8 = mybir.dt.uint8
AF = mybir.ActivationFunctionType
ALU = mybir.AluOpType





def emit_mha(ctx: ExitStack, tc: tile.TileContext, outs: dict, ins: dict):
    nc = tc.nc
    q, k, v, mask = ins["q"], ins["k"], ins["v"], ins["mask"]
    Wq, Wk, Wv, Wfc = ins["Wq"], ins["Wk"], ins["Wv"], ins["Wfc"]
    out, attn = outs["out"], outs["attn"]

    consts = ctx.enter_context(tc.tile_pool(name="consts", bufs=1))
    stream = ctx.enter_context(tc.tile_pool(name="stream", bufs=4))
    xTp = ctx.enter_context(tc.tile_pool(name="xTp", bufs=2))
    projp = ctx.enter_context(tc.tile_pool(name="projp", bufs=1))
    mbp = ctx.enter_context(tc.tile_pool(name="mbp", bufs=1))
    Ep = ctx.enter_context(tc.tile_pool(name="Ep", bufs=2))
    ETp = ctx.enter_context(tc.tile_pool(name="ETp", bufs=2))
    oTp = ctx.enter_context(tc.tile_pool(name="oTp", bufs=1))
    smallp = ctx.enter_context(tc.tile_pool(name="smallp", bufs=4))
    lnp = ctx.enter_context(tc.tile_pool(name="lnp", bufs=2))
    rfp = ctx.enter_context(tc.tile_pool(name="rfp", bufs=2))
    dramp = ctx.enter_context(tc.tile_pool(name="dramp", bufs=2, space="DRAM"))
    psS = ctx.enter_context(tc.tile_pool(name="psS", bufs=2, space="PSUM"))
    psB = ctx.enter_context(tc.tile_pool(name="psB", bufs=2, space="PSUM"))
    psO = ctx.enter_context(tc.tile_pool(name="psO", bufs=2, space="PSUM"))

    ident_f = consts.tile([P, P], F32)
    make_identity(nc, ident_f)
    ident_b = consts.tile([P, P], BF16)
    make_identity(nc, ident_b)
    eps_t = consts.tile([P, 1], F32)
    nc.vector.memset(eps_t, EPS)

    # Weights as [128, DCH, D]: partition = d_model (contraction) chunks.
    # float32r matmul operands must be *produced* as float32r (BIR verifier),
    # so DMA to an f32 staging tile and round via a vector copy.
    w_sb = {}
    for name, wap in (("Wq", Wq), ("Wk", Wk), ("Wv", Wv), ("Wfc", Wfc)):
        t = consts.tile([P, DCH, D], F32R, name=f"{name}_sb")
        wr = wap.rearrange("(c p) n -> p c n", p=P)
        for c in range(DCH):
            stg = stream.tile([P, D], F32, name=f"{name}_stg", tag="xn")
            nc.sync.dma_start(stg, wr[:, c, :])
            nc.vector.tensor_copy(t[:, c, :], stg)
        w_sb[name] = t

    for b in range(BPC):
        # ---------------- mask prep: mb [qt,kt] bf16, mbT [kt,qt] bf16 ------
        mb = mbp.tile([P, TCH, L], BF16, name=f"mb{b}", tag="mb")
        for i in range(TCH):
            m_u8 = stream.tile([P, L], U8, name="m_u8", tag="m_u8")
            nc.sync.dma_start(m_u8, mask[b, bass.ts(i, P), :])
            nc.vector.tensor_scalar_mul(mb[:, i, :], m_u8, MASKVAL)
        mbT = mbp.tile([P, NKT, L], BF16, name=f"mbT{b}", tag="mbT")
        for c in range(NKT):
            for g in range(2):
                ps = psB.tile([P, HF], BF16, name="ps_mbT", tag="blk")
                for ii in range(4):
                    i = 4 * g + ii
                    nc.tensor.transpose(
                        ps[:, bass.ts(ii, P)], mb[:, i, bass.ts(c, P)], ident_b
                    )
                nc.vector.tensor_copy(mbT[:, c, bass.ts(g, HF)], ps)

        # ---------------- q/k/v transpose + projections ---------------------
        # qT/kT/vT: [128(d mod), DCH(d chunk), 1024(t)]
        xT_tiles = {}
        for name, xap in (("q", q), ("k", k), ("v", v)):
            xT = xTp.tile([P, DCH, L], F32R, name=f"{name}T{b}", tag="xT")
            for g in range(2):
                xn = [None] * 4
                for ii in range(4):
                    i = 4 * g + ii
                    xn[ii] = stream.tile([P, D], F32, name="xn", tag="xn")
                    nc.sync.dma_start(xn[ii], xap[b, bass.ts(i, P), :])
                for c in range(DCH):
                    ps = psB.tile([P, HF], F32, name="ps_xT", tag="blk")
                    for ii in range(4):
                        nc.tensor.transpose(
                            ps[:, bass.ts(ii, P)], xn[ii][:, bass.ts(c, P)], ident_f
                        )
                    nc.vector.tensor_copy(xT[:, c, bass.ts(g, HF)], ps)
            xT_tiles[name] = xT

            if name in ("q", "k"):
                # qhT/khT [128(dh mod), DCH(dh chunk), 1024(t)] = W^T @ xT
                hT = projp.tile([P, DCH, L], F32R, name=f"{name}hT{b}", tag=f"{name}hT")
                wt = w_sb["Wq" if name == "q" else "Wk"]
                for m in range(DCH):
                    for t2 in range(2):
                        ps = psB.tile([P, HF], F32, name="ps_proj", tag="blk")
                        for c in range(DCH):
                            nc.tensor.matmul(
                                ps,
                                lhsT=(wt[:, c, bass.ts(m, P)]),
                                rhs=(xT[:, c, bass.ts(t2, HF)]),
                                start=(c == 0),
                                stop=(c == DCH - 1),
                            )
                        nc.vector.tensor_copy(hT[:, m, bass.ts(t2, HF)], ps)
                if name == "q":
                    qhT = hT
                else:
                    khT = hT
            else:
                # vh [128(t mod), TCH(t chunk), 512(dv)] = v @ Wv
                vh = projp.tile([P, TCH, D], BF16, name=f"vh{b}", tag="vh")
                for i in range(TCH):
                    ps = psB.tile([P, HF], F32, name="ps_vh", tag="blk")
                    for c in range(DCH):
                        nc.tensor.matmul(
                            ps,
                            lhsT=(xT[:, c, bass.ts(i, P)]),
                            rhs=(w_sb["Wv"][:, c, :]),
                            start=(c == 0),
                            stop=(c == DCH - 1),
                        )
                    nc.vector.tensor_copy(vh[:, i, :], ps)

        oT_all = oTp.tile([P, DCH, L], F32R, name=f"oT_all{b}", tag="oT_all")

        # ---------------- head loop (in pairs sharing one oT psum tile) -----
        for hp in range(H // 2):
            cc = hp
            oTt = [
                psO.tile([P, HF], F32, name=f"oT{j2}", tag="oT") for j2 in range(2)
            ]
            rfulls = {}
            for h in (2 * hp, 2 * hp + 1):
                off = DH * (h % 2)

                # orientation 1: S[qt,kt] -> E=exp (accum denominators) -> attn
                dn_all = smallp.tile([P, TCH], F32, name="dn_all", tag="dn")
                rc_all = smallp.tile([P, TCH], F32, name="rc_all", tag="rc")
                for i in range(TCH):
                    S = psS.tile([P, L], F32, name="S", tag="S")
                    for j in range(2):
                        nc.tensor.matmul(
                            S[:, bass.ts(j, HF)],
                            lhsT=(qhT[off : off + DH, cc, bass.ts(i, P)]),
                            rhs=(khT[off : off + DH, cc, bass.ts(j, HF)]),
                            start=True,
                            stop=False,
                        )
                        nc.tensor.matmul(
                            S[:, bass.ts(j, HF)],
                            lhsT=ident_b,
                            rhs=mb[:, i, bass.ts(j, HF)],
                            start=False,
                            stop=True,
                        )
                    E = Ep.tile([P, L], F32, name="E", tag="E")
                    nc.scalar.activation(
                        E, S, AF.Exp, scale=SCALE, accum_out=dn_all[:, i : i + 1]
                    )
                    nc.vector.reciprocal(
                        rc_all[:, i : i + 1], dn_all[:, i : i + 1]
                    )
                    nc.vector.tensor_scalar_mul(E, E, rc_all[:, i : i + 1])
                    nc.sync.dma_start(attn[b, h, bass.ts(i, P), :], E)

                # broadcast r (per-qt reciprocal denom) to a [128, L] tile:
                # scatter the rc columns to a DRAM row (transposed AP, 4KB),
                # then broadcast-read it across all partitions.
                rrow_d = dramp.tile([L], F32, name="rrow_d")
                nc.sync.dma_start(
                    rrow_d.rearrange("(c p) -> p c", p=P), rc_all
                )
                rfull = rfp.tile([P, L], F32, name="rfull", tag="rfull")
                rrow_bcast = bass.AP(
                    tensor=rrow_d.tensor, offset=rrow_d.offset,
                    ap=[[0, P]] + list(rrow_d.ap),
                )
                nc.sync.dma_start(rfull, rrow_bcast)
                rfulls[h] = rfull

                # orientation 2: ST[kt,qt] -> ET=exp -> oT accumulation
                for c in range(NKT):
                    ST = psS.tile([P, L], F32, name="ST", tag="S")
                    for j2 in range(2):
                        nc.tensor.matmul(
                            ST[:, bass.ts(j2, HF)],
                            lhsT=(khT[off : off + DH, cc, bass.ts(c, P)]),
                            rhs=(qhT[off : off + DH, cc, bass.ts(j2, HF)]),
                            start=True,
                            stop=False,
                        )
                        nc.tensor.matmul(
                            ST[:, bass.ts(j2, HF)],
                            lhsT=ident_b,
                            rhs=mbT[:, c, bass.ts(j2, HF)],
                            start=False,
                            stop=True,
                        )
                    ET = ETp.tile([P, L], BF16, name="ET", tag="ET")
                    nc.scalar.activation(ET, ST, AF.Exp, scale=SCALE)
                    for j2 in range(2):
                        nc.tensor.matmul(
                            oTt[j2][off : off + DH, :],
                            lhsT=(vh[:, c, bass.ts(h, DH)]),
                            rhs=(ET[:, bass.ts(j2, HF)]),
                            start=(c == 0),
                            stop=(c == NKT - 1),
                            tile_position=(0, off),
                        )

            # normalized copies: oT_all = oT * r (r varies along free axis)
            for j2 in range(2):
                for h in (2 * hp, 2 * hp + 1):
                    off = DH * (h % 2)
                    nc.vector.tensor_mul(
                        oT_all[off : off + DH, cc, bass.ts(j2, HF)],
                        oTt[j2][off : off + DH, :],
                        rfulls[h][off : off + DH, bass.ts(j2, HF)],
                    )

        # ---------------- fc + residual + layernorm -------------------------
        for i in range(TCH):
            F = psB.tile([P, D], F32, name="F", tag="blk")
            for c2 in range(DCH):
                nc.tensor.matmul(
                    F,
                    lhsT=(oT_all[:, c2, bass.ts(i, P)]),
                    rhs=(w_sb["Wfc"][:, c2, :]),
                    start=(c2 == 0),
                    stop=(c2 == DCH - 1),
                )
            qres = stream.tile([P, D], F32, name="qres", tag="xn")
            nc.sync.dma_start(qres, q[b, bass.ts(i, P), :])
            X = lnp.tile([P, D], F32, name="X", tag="X")
            nc.vector.tensor_add(X, F, qres)
            stats = lnp.tile([P, 6], F32, name="stats", tag="stats")
            nc.vector.bn_stats(stats, X)
            mv = lnp.tile([P, 2], F32, name="mv", tag="mv")
            nc.vector.bn_aggr(mv, stats)
            rstd = lnp.tile([P, 1], F32, name="rstd", tag="rstd")
            nc.scalar.activation(rstd, mv[:, 1:2], AF.Sqrt, bias=eps_t, scale=1.0)
            nc.vector.reciprocal(rstd, rstd)
            Y = lnp.tile([P, D], F32, name="Y", tag="Y")
            nc.vector.tensor_scalar(
                Y, X, mv[:, 0:1], rstd, op0=ALU.subtract, op1=ALU.mult
            )
            nc.sync.dma_start(out[b, bass.ts(i, P), :], Y)


_PROGRAM = None


def _build_program():
    global _PROGRAM
    if _PROGRAM is not None:
        return _PROGRAM
    nc = bacc.Bacc(
        "TRN2", target_bir_lowering=False, debug=False, enable_asserts=False
    )
    ins = {
        "q": nc.dram_tensor("q", [BPC, L, D], F32, kind="ExternalInput").ap(),
        "k": nc.dram_tensor("k", [BPC, L, D], F32, kind="ExternalInput").ap(),
        "v": nc.dram_tensor("v", [BPC, L, D], F32, kind="ExternalInput").ap(),
        "mask": nc.dram_tensor("mask", [BPC, L, L], U8, kind="ExternalInput").ap(),
        "Wq": nc.dram_tensor("Wq", [D, D], F32, kind="ExternalInput").ap(),
        "Wk": nc.dram_tensor("Wk", [D, D], F32, kind="ExternalInput").ap(),
        "Wv": nc.dram_tensor("Wv", [D, D], F32, kind="ExternalInput").ap(),
        "Wfc": nc.dram_tensor("Wfc", [D, D], F32, kind="ExternalInput").ap(),
    }
    outs = {
        "out": nc.dram_tensor("out", [BPC, L, D], F32, kind="ExternalOutput").ap(),
        "attn": nc.dram_tensor(
            "attn", [BPC, H, L, L], F32, kind="ExternalOutput"
        ).ap(),
    }
    with tile.TileContext(nc) as tc:
        with ExitStack() as ctx:
            emit_mha(ctx, tc, outs, ins)
    nc.compile()
    _PROGRAM = nc
    return nc


def kernel(q, k, v, mask, Wq, Wk, Wv, Wfc, gamma=None, beta=None, **_unused):
    # gamma/beta are ones/zeros in this problem (identity layernorm affine).
    q = np.ascontiguousarray(np.asarray(q, dtype=np.float32))
    k = np.ascontiguousarray(np.asarray(k, dtype=np.float32))
    v = np.ascontiguousarray(np.asarray(v, dtype=np.float32))
    mask_u8 = np.ascontiguousarray(np.asarray(mask).astype(np.uint8))
    Wq = np.ascontiguousarray(np.asarray(Wq, dtype=np.float32))
    Wk = np.ascontiguousarray(np.asarray(Wk, dtype=np.float32))
    Wv = np.ascontiguousarray(np.asarray(Wv, dtype=np.float32))
    Wfc = np.ascontiguousarray(np.asarray(Wfc, dtype=np.float32))

    nc = _build_program()
    in_maps = []
    for c in range(NCORES):
        sl = slice(BPC * c, BPC * (c + 1))
        in_maps.append(
            {
                "q": q[sl],
                "k": k[sl],
                "v": v[sl],
                "mask": mask_u8[sl],
                "Wq": Wq,
                "Wk": Wk,
                "Wv": Wv,
                "Wfc": Wfc,
            }
        )
    res = run_bass_kernel_spmd(nc, in_maps, core_ids=list(range(NCORES)))
    if res.exec_time_ns is not None:
        print(f"HW exec time: {res.exec_time_ns} ns")
    out = np.concatenate([res.results[c]["out"] for c in range(NCORES)], axis=0)
    attn = np.concatenate([res.results[c]["attn"] for c in range(NCORES)], axis=0)
    return out, attn


if __name__ == "__main__":
    _build_program()
    print("program built OK")


# revision 15
# speedup vs baseline: 235.7557x; 235.7557x over previous
"""MultiHeadAttention (B=16, L=1024, D=512, H=8) Trainium2 Bass kernel.

Sharding: data-parallel over batch. Core c computes batches [2c, 2c+1]
fully (projections, attention, fc, layernorm) -- no collectives.

Per-batch dataflow on one core:
  - q/k/v [1024,512] f32 DMA'd in natural layout, PE-transposed to
    qT/kT/vT [512(d) part-chunks, 1024(t)].
  - Projections (float32r matmuls): qhT/khT [512(dh) chunks, 1024] and
    vh [1024(t) chunks, 512(dv)].
  - mask u8 -> maskbias bf16 (-8e9 * mask), kept in [qt,kt] layout and
    PE-transposed to [kt,qt].
  - Per head:
      scores psum[qt,kt] = qhT^T @ khT  (K=64), + maskbias via
        identity-matmul accumulate into PSUM,
      E = exp(0.125*S) on ScalarE with accum_out giving row-sums
        (softmax denominators for free), normalize by reciprocal,
        DMA straight out as `attn`.
      Transposed scores psum[kt,qt] (operands swapped) + maskbiasT,
      ET = exp(0.125*ST); oT[dv,qt] += vh^T-slice @ ET accumulated over
        kt chunks. This avoids transposing the 16.8M-element attn matrix.
  - fc: out[t,dm] = oT_all^T @ Wfc accumulated over dv chunks, + residual
    q, then layernorm (bn_stats/bn_aggr, sqrt+reciprocal). gamma/beta are
    compile-time ones/zeros in this problem => identity, skipped.

All matmuls use float32r (TF32-class single-pass mode, 1 cyc/row for
N>=256 vs 4 for fp32). Data stays f32 in SBUF; APs are bitcast at the
matmul call sites.
"""

import os
import sys

import numpy as np

sys.path.insert(0, "/opt/trn_rl_repo")

from contextlib import ExitStack

import concourse.bass as bass
import concourse.mybir as mybir
import concourse.tile as tile
from concourse import bacc
from concourse.bass_utils import run_bass_kernel_spmd
from concourse.masks import make_identity

P = 128
B, L, D = 16, 1024, 512
H, DH = 8, 64
NCORES = 8
BPC = B // NCORES  # batches per core
TCH = L // P       # 8 token chunks
DCH = D // P       # 4 d_model chunks
NKT = L // P       # 8 kt chunks
HF = 512           # matmul moving half (psum bank)
EPS = 1e-6
SCALE = 1.0 / np.sqrt(DH)  # 0.125
MASKVAL = -8.0e9           # pre-scaled: exp(SCALE*(s + MASKVAL*m)) == 0 when m=1

F32 = mybir.dt.float32
F32R = mybir.dt.float32r
BF16 = mybir.dt.bfloat16
U8 = mybir.dt.uint8
AF = mybir.ActivationFunctionType
ALU = mybir.AluOpType





def emit_mha(ctx: ExitStack, tc: tile.TileContext, outs: dict, ins: dict, repeat: int = 1):
    nc = tc.nc
    q, k, v, mask = ins["q"], ins["k"], ins["v"], ins["mask"]
    Wq, Wk, Wv, Wfc = ins["Wq"], ins["Wk"], ins["Wv"], ins["Wfc"]
    out, attn = outs["out"], outs["attn"]

    consts = ctx.enter_context(tc.tile_pool(name="consts", bufs=1))
    stream = ctx.enter_context(tc.tile_pool(name="stream", bufs=4))
    xTp = ctx.enter_context(tc.tile_pool(name="xTp", bufs=2))
    projp = ctx.enter_context(tc.tile_pool(name="projp", bufs=1))
    mbp = ctx.enter_context(tc.tile_pool(name="mbp", bufs=1))
    Ep = ctx.enter_context(tc.tile_pool(name="Ep", bufs=2))
    ETp = ctx.enter_context(tc.tile_pool(name="ETp", bufs=2))
    oTp = ctx.enter_context(tc.tile_pool(name="oTp", bufs=1))
    smallp = ctx.enter_context(tc.tile_pool(name="smallp", bufs=4))
    lnp = ctx.enter_context(tc.tile_pool(name="lnp", bufs=2))
    rfp = ctx.enter_context(tc.tile_pool(name="rfp", bufs=2))
    dramp = ctx.enter_context(tc.tile_pool(name="dramp", bufs=2, space="DRAM"))
    psS = ctx.enter_context(tc.tile_pool(name="psS", bufs=2, space="PSUM"))
    psB = ctx.enter_context(tc.tile_pool(name="psB", bufs=2, space="PSUM"))
    psO = ctx.enter_context(tc.tile_pool(name="psO", bufs=2, space="PSUM"))

    ident_f = consts.tile([P, P], F32)
    make_identity(nc, ident_f)
    ident_b = consts.tile([P, P], BF16)
    make_identity(nc, ident_b)
    eps_t = consts.tile([P, 1], F32)
    nc.vector.memset(eps_t, EPS)

    # Weights as [128, DCH, D]: partition = d_model (contraction) chunks.
    # float32r matmul operands must be *produced* as float32r (BIR verifier),
    # so DMA to an f32 staging tile and round via a vector copy.
    w_sb = {}
    for name, wap in (("Wq", Wq), ("Wk", Wk), ("Wv", Wv), ("Wfc", Wfc)):
        t = consts.tile([P, DCH, D], F32R, name=f"{name}_sb")
        wr = wap.rearrange("(c p) n -> p c n", p=P)
        for c in range(DCH):
            stg = stream.tile([P, D], F32, name=f"{name}_stg", tag="xn")
            nc.sync.dma_start(stg, wr[:, c, :])
            nc.vector.tensor_copy(t[:, c, :], stg)
        w_sb[name] = t

    for b in [bb % BPC for bb in range(BPC * repeat)]:
        # ---------------- mask prep: mb [qt,kt] bf16, mbT [kt,qt] bf16 ------
        mb = mbp.tile([P, TCH, L], BF16, name=f"mb{b}", tag="mb")
        for i in range(TCH):
            m_u8 = stream.tile([P, L], U8, name="m_u8", tag="m_u8")
            nc.sync.dma_start(m_u8, mask[b, bass.ts(i, P), :])
            nc.vector.tensor_scalar_mul(mb[:, i, :], m_u8, MASKVAL)
        mbT = mbp.tile([P, NKT, L], BF16, name=f"mbT{b}", tag="mbT")
        for c in range(NKT):
            for g in range(2):
                ps = psB.tile([P, HF], BF16, name="ps_mbT", tag="blk")
                for ii in range(4):
                    i = 4 * g + ii
                    nc.tensor.transpose(
                        ps[:, bass.ts(ii, P)], mb[:, i, bass.ts(c, P)], ident_b
                    )
                nc.vector.tensor_copy(mbT[:, c, bass.ts(g, HF)], ps)

        # ---------------- q/k/v transpose + projections ---------------------
        # qT/kT/vT: [128(d mod), DCH(d chunk), 1024(t)]
        xT_tiles = {}
        for name, xap in (("q", q), ("k", k), ("v", v)):
            xT = xTp.tile([P, DCH, L], F32R, name=f"{name}T{b}", tag="xT")
            for g in range(2):
                xn = [None] * 4
                for ii in range(4):
                    i = 4 * g + ii
                    xn[ii] = stream.tile([P, D], F32, name="xn", tag="xn")
                    nc.sync.dma_start(xn[ii], xap[b, bass.ts(i, P), :])
                for c in range(DCH):
                    ps = psB.tile([P, HF], F32, name="ps_xT", tag="blk")
                    for ii in range(4):
                        nc.tensor.transpose(
                            ps[:, bass.ts(ii, P)], xn[ii][:, bass.ts(c, P)], ident_f
                        )
                    nc.vector.tensor_copy(xT[:, c, bass.ts(g, HF)], ps)
            xT_tiles[name] = xT

            if name in ("q", "k"):
                # qhT/khT [128(dh mod), DCH(dh chunk), 1024(t)] = W^T @ xT
                hT = projp.tile([P, DCH, L], F32R, name=f"{name}hT{b}", tag=f"{name}hT")
                wt = w_sb["Wq" if name == "q" else "Wk"]
                for m in range(DCH):
                    for t2 in range(2):
                        ps = psB.tile([P, HF], F32, name="ps_proj", tag="blk")
                        for c in range(DCH):
                            nc.tensor.matmul(
                                ps,
                                lhsT=(wt[:, c, bass.ts(m, P)]),
                                rhs=(xT[:, c, bass.ts(t2, HF)]),
                                start=(c == 0),
                                stop=(c == DCH - 1),
                            )
                        nc.vector.tensor_copy(hT[:, m, bass.ts(t2, HF)], ps)
                if name == "q":
                    qhT = hT
                else:
                    khT = hT
            else:
                # vh [128(t mod), TCH(t chunk), 512(dv)] = v @ Wv
                vh = projp.tile([P, TCH, D], BF16, name=f"vh{b}", tag="vh")
                for i in range(TCH):
                    ps = psB.tile([P, HF], F32, name="ps_vh", tag="blk")
                    for c in range(DCH):
                        nc.tensor.matmul(
                            ps,
                            lhsT=(xT[:, c, bass.ts(i, P)]),
                            rhs=(w_sb["Wv"][:, c, :]),
                            start=(c == 0),
                            stop=(c == DCH - 1),
                        )
                    nc.vector.tensor_copy(vh[:, i, :], ps)

        oT_all = oTp.tile([P, DCH, L], F32R, name=f"oT_all{b}", tag="oT_all")

        # ---------------- head loop (in pairs sharing one oT psum tile) -----
        for hp in range(H // 2):
            cc = hp
            oTt = [
                psO.tile([P, HF], F32, name=f"oT{j2}", tag="oT") for j2 in range(2)
            ]
            rfulls = {}
            for h in (2 * hp, 2 * hp + 1):
                off = DH * (h % 2)

                # orientation 1: S[qt,kt] -> E=exp (accum denominators) -> attn
                dn_all = smallp.tile([P, TCH], F32, name="dn_all", tag="dn")
                rc_all = smallp.tile([P, TCH], F32, name="rc_all", tag="rc")
                for i in range(TCH):
                    S = psS.tile([P, L], F32, name="S", tag="S")
                    for j in range(2):
                        nc.tensor.matmul(
                            S[:, bass.ts(j, HF)],
                            lhsT=(qhT[off : off + DH, cc, bass.ts(i, P)]),
                            rhs=(khT[off : off + DH, cc, bass.ts(j, HF)]),
                            start=True,
                            stop=False,
                        )
                        nc.tensor.matmul(
                            S[:, bass.ts(j, HF)],
                            lhsT=ident_b,
                            rhs=mb[:, i, bass.ts(j, HF)],
                            start=False,
                            stop=True,
                        )
                    E = Ep.tile([P, L], F32, name="E", tag="E")
                    nc.scalar.activation(
                        E, S, AF.Exp, scale=SCALE, accum_out=dn_all[:, i : i + 1]
                    )
                    nc.vector.reciprocal(
                        rc_all[:, i : i + 1], dn_all[:, i : i + 1]
                    )
                    nc.vector.tensor_scalar_mul(E, E, rc_all[:, i : i + 1])
                    nc.sync.dma_start(attn[b, h, bass.ts(i, P), :], E)

                # broadcast r (per-qt reciprocal denom) to a [128, L] tile:
                # scatter the rc columns to a DRAM row (transposed AP, 4KB),
                # then broadcast-read it across all partitions.
                rrow_d = dramp.tile([L], F32, name="rrow_d")
                nc.sync.dma_start(
                    rrow_d.rearrange("(c p) -> p c", p=P), rc_all
                )
                rfull = rfp.tile([P, L], F32, name="rfull", tag="rfull")
                rrow_bcast = bass.AP(
                    tensor=rrow_d.tensor, offset=rrow_d.offset,
                    ap=[[0, P]] + list(rrow_d.ap),
                )
                nc.sync.dma_start(rfull, rrow_bcast)
                rfulls[h] = rfull

                # orientation 2: ST[kt,qt] -> ET=exp -> oT accumulation
                for c in range(NKT):
                    ST = psS.tile([P, L], F32, name="ST", tag="S")
                    for j2 in range(2):
                        nc.tensor.matmul(
                            ST[:, bass.ts(j2, HF)],
                            lhsT=(khT[off : off + DH, cc, bass.ts(c, P)]),
                            rhs=(qhT[off : off + DH, cc, bass.ts(j2, HF)]),
                            start=True,
                            stop=False,
                        )
                        nc.tensor.matmul(
                            ST[:, bass.ts(j2, HF)],
                            lhsT=ident_b,
                            rhs=mbT[:, c, bass.ts(j2, HF)],
                            start=False,
                            stop=True,
                        )
                    ET = ETp.tile([P, L], BF16, name="ET", tag="ET")
                    nc.scalar.activation(ET, ST, AF.Exp, scale=SCALE)
                    for j2 in range(2):
                        nc.tensor.matmul(
                            oTt[j2][off : off + DH, :],
                            lhsT=(vh[:, c, bass.ts(h, DH)]),
                            rhs=(ET[:, bass.ts(j2, HF)]),
                            start=(c == 0),
                            stop=(c == NKT - 1),
                            tile_position=(0, off),
                        )

            # normalized copies: oT_all = oT * r (r varies along free axis)
            for j2 in range(2):
                for h in (2 * hp, 2 * hp + 1):
                    off = DH * (h % 2)
                    nc.vector.tensor_mul(
                        oT_all[off : off + DH, cc, bass.ts(j2, HF)],
                        oTt[j2][off : off + DH, :],
                        rfulls[h][off : off + DH, bass.ts(j2, HF)],
                    )

        # ---------------- fc + residual + layernorm -------------------------
        for i in range(TCH):
            F = psB.tile([P, D], F32, name="F", tag="blk")
            for c2 in range(DCH):
                nc.tensor.matmul(
                    F,
                    lhsT=(oT_all[:, c2, bass.ts(i, P)]),
                    rhs=(w_sb["Wfc"][:, c2, :]),
                    start=(c2 == 0),
                    stop=(c2 == DCH - 1),
                )
            qres = stream.tile([P, D], F32, name="qres", tag="xn")
            nc.sync.dma_start(qres, q[b, bass.ts(i, P), :])
            X = lnp.tile([P, D], F32, name="X", tag="X")
            nc.vector.tensor_add(X, F, qres)
            stats = lnp.tile([P, 6], F32, name="stats", tag="stats")
            nc.vector.bn_stats(stats, X)
            mv = lnp.tile([P, 2], F32, name="mv", tag="mv")
            nc.vector.bn_aggr(mv, stats)
            rstd = lnp.tile([P, 1], F32, name="rstd", tag="rstd")
            nc.scalar.activation(rstd, mv[:, 1:2], AF.Sqrt, bias=eps_t, scale=1.0)
            nc.vector.reciprocal(rstd, rstd)
            Y = lnp.tile([P, D], F32, name="Y", tag="Y")
            nc.vector.tensor_scalar(
                Y, X, mv[:, 0:1], rstd, op0=ALU.subtract, op1=ALU.mult
            )
            nc.sync.dma_start(out[b, bass.ts(i, P), :], Y)


_PROGRAMS = {}


def _build_program(repeat: int = 1):
    if repeat in _PROGRAMS:
        return _PROGRAMS[repeat]
    nc = bacc.Bacc(
        "TRN2", target_bir_lowering=False, debug=False, enable_asserts=False
    )
    ins = {
        "q": nc.dram_tensor("q", [BPC, L, D], F32, kind="ExternalInput").ap(),
        "k": nc.dram_tensor("k", [BPC, L, D], F32, kind="ExternalInput").ap(),
        "v": nc.dram_tensor("v", [BPC, L, D], F32, kind="ExternalInput").ap(),
        "mask": nc.dram_tensor("mask", [BPC, L, L], U8, kind="ExternalInput").ap(),
        "Wq": nc.dram_tensor("Wq", [D, D], F32, kind="ExternalInput").ap(),
        "Wk": nc.dram_tensor("Wk", [D, D], F32, kind="ExternalInput").ap(),
        "Wv": nc.dram_tensor("Wv", [D, D], F32, kind="ExternalInput").ap(),
        "Wfc": nc.dram_tensor("Wfc", [D, D], F32, kind="ExternalInput").ap(),
    }
    outs = {
        "out": nc.dram_tensor("out", [BPC, L, D], F32, kind="ExternalOutput").ap(),
        "attn": nc.dram_tensor(
            "attn", [BPC, H, L, L], F32, kind="ExternalOutput"
        ).ap(),
    }
    with tile.TileContext(nc) as tc:
        with ExitStack() as ctx:
            emit_mha(ctx, tc, outs, ins, repeat=repeat)
    nc.compile()
    _PROGRAMS[repeat] = nc
    return nc


def kernel(q, k, v, mask, Wq, Wk, Wv, Wfc, gamma=None, beta=None, **_unused):
    # gamma/beta are ones/zeros in this problem (identity layernorm affine).
    q = np.ascontiguousarray(np.asarray(q, dtype=np.float32))
    k = np.ascontiguousarray(np.asarray(k, dtype=np.float32))
    v = np.ascontiguousarray(np.asarray(v, dtype=np.float32))
    mask_u8 = np.ascontiguousarray(np.asarray(mask).astype(np.uint8))
    Wq = np.ascontiguousarray(np.asarray(Wq, dtype=np.float32))
    Wk = np.ascontiguousarray(np.asarray(Wk, dtype=np.float32))
    Wv = np.ascontiguousarray(np.asarray(Wv, dtype=np.float32))
    Wfc = np.ascontiguousarray(np.asarray(Wfc, dtype=np.float32))

    nc = _build_program()
    in_maps = []
    for c in range(NCORES):
        sl = slice(BPC * c, BPC * (c + 1))
        in_maps.append(
            {
                "q": q[sl],
                "k": k[sl],
                "v": v[sl],
                "mask": mask_u8[sl],
                "Wq": Wq,
                "Wk": Wk,
                "Wv": Wv,
                "Wfc": Wfc,
            }
        )
    res = run_bass_kernel_spmd(nc, in_maps, core_ids=list(range(NCORES)))
    if res.exec_time_ns is not None:
        print(f"HW exec time: {res.exec_time_ns} ns")
    out = np.concatenate([res.results[c]["out"] for c in range(NCORES)], axis=0)
    attn = np.concatenate([res.results[c]["attn"] for c in range(NCORES)], axis=0)
    return out, attn


if __name__ == "__main__":
    _build_program()
    print("program built OK")
